# revision 18
# baseline (speedup 1.0000x reference)
"""Trainium2 Bass kernel for the 6-direction vision-Mamba block.

Sharding: 24 uniform slots = (6 directions x 2 batch) x (2 halves of d_inner),
3 slots per core on 8 cores. Host pre-permutes channels per slot so the device
program is identical across cores (SPMD); each slot computes the full in_proj /
conv / x_proj (needed for the full-d_inner contraction feeding dt/B/C) and the
scan + out_proj for its 512-channel half.  Host applies the direction
permutations, sums the 24 partial out_proj results and divides by 6.
"""

import os
import sys

for _p in ("/opt/trn_rl_repo",):
    if os.path.isdir(_p) and _p not in sys.path:
        sys.path.insert(0, _p)

import numpy as np

# ---------------------------------------------------------------- constants
DM = 512       # d_model
DI = 1024      # d_inner
DS = 16        # d_state
DTR = 32       # dt_rank
DCONV = 4
B, D, H, W = 2, 10, 10, 10
L = D * H * W  # 1000
NCORE = 8
NSLOT = 3          # slots per core
NCH = 512          # scan channels per slot
NBLK = NCH // 128  # 4 scan blocks per slot
CBLK = DI // 128   # 8 conv blocks (full d_inner)
LH = 2             # l-halves for matmul N=500
LN = L // LH       # 500

# tuning knobs
SGRP = 2            # states per scan group (16/SGRP groups)
NG = DS // SGRP
STREAM_FP32 = False  # dA/dBu/h/prod/w in fp32 instead of bf16
BC_FP32 = False      # B_rep/C_rep dtype

_CACHE = {}
LAST_RESULTS = None


# ---------------------------------------------------------------- host: perms
def _direction_perms():
    """perm[i][l] = canonical flat (D,H,W) index of the l-th token of dir i."""
    idx = np.arange(L, dtype=np.int64).reshape(D, H, W)
    p1 = idx.reshape(-1)
    p2 = idx.transpose(0, 2, 1).reshape(-1)          # (d, w, h) order
    p3 = np.rot90(idx, 1, axes=(0, 2)).reshape(-1)   # rot90 in (D, W)
    return [p1, p2, p3, p1[::-1].copy(), p2[::-1].copy(), p3[::-1].copy()]


def _slot_params(u, h, inputs, perms):
    """Host-side tensors for one slot (unit u = (dir, batch), half h)."""
    d = u // 2
    b = u % 2
    my = np.arange(h * NCH, (h + 1) * NCH)
    other = np.arange((1 - h) * NCH, (2 - h) * NCH)
    chperm = np.concatenate([my, other])          # my half first

    x = inputs["x"].reshape(B, DM, L)
    x_seq = np.ascontiguousarray(x[b][:, perms[d]], dtype=np.float32)

    ipw = inputs["in_proj_w"][d]                  # (2*DI, DM)
    ipw_sl = np.concatenate([ipw[chperm], ipw[DI + my]], axis=0)   # (1536, DM)
    ipw_t = np.ascontiguousarray(ipw_sl.T, dtype=np.float32)       # (512, 1536)

    cw = inputs["conv_w"][d][chperm, 0, :]        # (1024, 4)
    cdiag = np.zeros((CBLK, DCONV, 128, 128), dtype=np.float32)
    r = np.arange(128)
    for blk in range(CBLK):
        for k in range(DCONV):
            cdiag[blk, k, r, r] = cw[blk * 128 + r, k]
    cb = inputs["conv_b"][d][chperm].reshape(CBLK, 128).T          # (128, 8)
    cb = np.ascontiguousarray(cb, dtype=np.float32)

    xpw = inputs["x_proj_w"][d]                   # (64, DI)
    xpw_t = np.ascontiguousarray(xpw[:, chperm].T, dtype=np.float32)  # (1024, 64)

    dpw = inputs["dt_proj_w"][d][my]              # (512, 32)
    dpw_t = np.ascontiguousarray(dpw.T, dtype=np.float32)             # (32, 512)
    dpb = inputs["dt_proj_b"][d][my].reshape(NBLK, 128).T
    dpb = np.ascontiguousarray(dpb, dtype=np.float32)                 # (128, 4)

    A = -np.exp(inputs["A_log"][d][my])           # (512, 16)
    A_sb = A.reshape(NBLK, 128, DS).transpose(1, 0, 2).reshape(128, NBLK * DS)
    A_sb = np.ascontiguousarray(A_sb, dtype=np.float32)               # (128, 64)

    Dp = inputs["D_param"][d][my].reshape(NBLK, 128).T
    Dp = np.ascontiguousarray(Dp, dtype=np.float32)                   # (128, 4)

    opw = inputs["out_proj_w"][d]                 # (DM, DI)
    opw_t = np.ascontiguousarray(opw[:, my].T, dtype=np.float32)      # (512, 512)

    return dict(x=x_seq, ipw=ipw_t, cdiag=cdiag, cb=cb, xpw=xpw_t,
                dpw=dpw_t, dpb=dpb, A=A_sb, Dp=Dp, opw=opw_t,
                meta=(d, b, h))


# ---------------------------------------------------------------- device build
def _build_program():
    import concourse.bass as bass
    import concourse.bacc as bacc
    import concourse.tile as tile
    from concourse import mybir
    from contextlib import ExitStack

    f32 = mybir.dt.float32
    sdt = f32 if STREAM_FP32 else mybir.dt.bfloat16
    bcdt = f32 if BC_FP32 else mybir.dt.bfloat16
    AF = mybir.ActivationFunctionType
    OP = mybir.AluOpType

    nc = bacc.Bacc("TRN2", target_bir_lowering=False, debug=False,
                   enable_asserts=False, num_devices=1)

    ins = []
    outs = []
    bcs = []
    for si in range(NSLOT):
        t = {
            "x": nc.dram_tensor(f"x_{si}", [DM, L], f32, kind="ExternalInput"),
            "ipw": nc.dram_tensor(f"ipw_{si}", [DM, DI + NCH], f32, kind="ExternalInput"),
            "cdiag": nc.dram_tensor(f"cdiag_{si}", [CBLK, DCONV, 128, 128], f32, kind="ExternalInput"),
            "cb": nc.dram_tensor(f"cb_{si}", [128, CBLK], f32, kind="ExternalInput"),
            "xpw": nc.dram_tensor(f"xpw_{si}", [DI, DTR + 2 * DS], f32, kind="ExternalInput"),
            "dpw": nc.dram_tensor(f"dpw_{si}", [DTR, NCH], f32, kind="ExternalInput"),
            "dpb": nc.dram_tensor(f"dpb_{si}", [128, NBLK], f32, kind="ExternalInput"),
            "A": nc.dram_tensor(f"A_{si}", [128, NBLK * DS], f32, kind="ExternalInput"),
            "Dp": nc.dram_tensor(f"Dp_{si}", [128, NBLK], f32, kind="ExternalInput"),
            "opw": nc.dram_tensor(f"opw_{si}", [NCH, DM], f32, kind="ExternalInput"),
        }
        ins.append(t)
        outs.append(nc.dram_tensor(f"yout_{si}", [DM, L], f32, kind="ExternalOutput"))
        bcs.append(nc.dram_tensor(f"bc_{si}", [2 * DS, L], bcdt))  # Internal

    with tile.TileContext(nc) as tc, ExitStack() as ctx:
        p_x = ctx.enter_context(tc.tile_pool(name="p_x", bufs=4))
        p_w = ctx.enter_context(tc.tile_pool(name="p_w", bufs=4))
        p_xcp = ctx.enter_context(tc.tile_pool(name="p_xcp", bufs=2))
        p_xt = ctx.enter_context(tc.tile_pool(name="p_xt", bufs=8))
        p_z = ctx.enter_context(tc.tile_pool(name="p_z", bufs=4))
        p_dt = ctx.enter_context(tc.tile_pool(name="p_dt", bufs=4))
        p_w2 = ctx.enter_context(tc.tile_pool(name="p_w2", bufs=4))
        p_xdbl = ctx.enter_context(tc.tile_pool(name="p_xdbl", bufs=1))
        p_bc = ctx.enter_context(tc.tile_pool(name="p_bc", bufs=2))
        p_scan = ctx.enter_context(tc.tile_pool(name="p_scan", bufs=2))
        p_yacc = ctx.enter_context(tc.tile_pool(name="p_yacc", bufs=4))
        p_misc = ctx.enter_context(tc.tile_pool(name="p_misc", bufs=2))
        p_yo = ctx.enter_context(tc.tile_pool(name="p_yo", bufs=1))
        p_yg = ctx.enter_context(tc.tile_pool(name="p_yg", bufs=4))
        p_const = ctx.enter_context(tc.tile_pool(name="p_const", bufs=4))
        p_ps = ctx.enter_context(tc.tile_pool(name="p_ps", bufs=3, space="PSUM"))
        p_psx = ctx.enter_context(tc.tile_pool(name="p_psx", bufs=2, space="PSUM"))

        for si in range(NSLOT):
            T = ins[si]
            # ---- constants
            cb_sb = p_const.tile([128, CBLK], f32, tag="cb")
            nc.sync.dma_start(cb_sb[:, :], T["cb"].ap())
            dpb_sb = p_const.tile([128, NBLK], f32, tag="dpb")
            nc.sync.dma_start(dpb_sb[:, :], T["dpb"].ap())
            A_sb = p_const.tile([128, NBLK * DS], f32, tag="A")
            nc.sync.dma_start(A_sb[:, :], T["A"].ap())
            Dp_sb = p_const.tile([128, NBLK], f32, tag="Dp")
            nc.sync.dma_start(Dp_sb[:, :], T["Dp"].ap())
            cbh_sb = p_const.tile([128, CBLK], f32, tag="cbh")
            nc.vector.tensor_scalar_mul(cbh_sb[:, :], cb_sb[:, :], 0.5)

            # ---- load x (4 k-tiles)
            x_sb = []
            for k in range(4):
                xin = p_x.tile([128, L], f32, tag="x")
                nc.sync.dma_start(xin[:, :], T["x"].ap()[k * 128:(k + 1) * 128, :])
                x_sb.append(xin)

            # ---- in_proj xc rows (8 blocks) interleaved with depthwise conv
            xt_all = []
            for blk in range(CBLK):
                xcp = p_xcp.tile([128, L + 3], f32, tag="xcp")
                nc.gpsimd.memset(xcp[:, 0:3], 0.0)
                lw = [p_w.tile([128, 128], f32, tag="lw_ip", name="lw_ip") for _ in range(4)]
                for k in range(4):
                    nc.sync.dma_start(
                        lw[k][:, :],
                        T["ipw"].ap()[k * 128:(k + 1) * 128,
                                      blk * 128:(blk + 1) * 128])
                for lh in range(LH):
                    ps = p_ps.tile([128, LN], f32, tag="mm")
                    for k in range(4):
                        nc.tensor.matmul(ps[:, :], lw[k][:, :],
                                         x_sb[k][:, lh * LN:(lh + 1) * LN],
                                         start=(k == 0), stop=(k == 3))
                    nc.vector.tensor_copy(xcp[:, 3 + lh * LN: 3 + (lh + 1) * LN],
                                          ps[:, :])
                # depthwise conv via 4 diagonal matmuls + fused bias+silu
                xt_t = p_xt.tile([128, L], f32, tag="xt")
                lwc = [p_w.tile([128, 128], f32, tag="lw_cv", name="lw_cv") for _ in range(4)]
                for k in range(4):
                    nc.sync.dma_start(lwc[k][:, :], T["cdiag"].ap()[blk, k])
                for lh in range(LH):
                    psc = p_ps.tile([128, LN], f32, tag="mm")
                    for k in range(4):
                        nc.tensor.matmul(psc[:, :], lwc[k][:, :],
                                         xcp[:, lh * LN + k: lh * LN + k + LN],
                                         start=(k == 0), stop=(k == 3))
                    # silu(v+cb) = (v+cb)*(0.5 + 0.5*tanh((v+cb)/2))
                    cvt = p_misc.tile([128, LN], sdt, tag="cvt")
                    nc.scalar.activation(cvt[:, :], psc[:, :], AF.Tanh,
                                         bias=cbh_sb[:, blk:blk + 1], scale=0.5)
                    cvt2 = p_misc.tile([128, LN], sdt, tag="cvt2")
                    nc.vector.tensor_scalar(cvt2[:, :], cvt[:, :], 0.5, 0.5,
                                            OP.mult, OP.add)
                    nc.vector.scalar_tensor_tensor(
                        xt_t[:, lh * LN:(lh + 1) * LN], psc[:, :],
                        cb_sb[:, blk:blk + 1], cvt2[:, :], OP.add, OP.mult)
                xt_all.append(xt_t)

            # ---- in_proj z rows (4 blocks, my half)
            z_all = []
            for zb in range(NBLK):
                i = CBLK + zb
                lw = [p_w.tile([128, 128], f32, tag="lw_ip", name="lw_ip") for _ in range(4)]
                for k in range(4):
                    nc.sync.dma_start(
                        lw[k][:, :],
                        T["ipw"].ap()[k * 128:(k + 1) * 128,
                                      i * 128:(i + 1) * 128])
                z_t = p_z.tile([128, L], sdt, tag="z")
                for lh in range(LH):
                    ps = p_ps.tile([128, LN], f32, tag="mm")
                    for k in range(4):
                        nc.tensor.matmul(ps[:, :], lw[k][:, :],
                                         x_sb[k][:, lh * LN:(lh + 1) * LN],
                                         start=(k == 0), stop=(k == 3))
                    nc.scalar.copy(z_t[:, lh * LN:(lh + 1) * LN], ps[:, :])
                z_all.append(z_t)

            # ---- x_proj (contract full d_inner)
            lwx = [p_w.tile([128, DTR + 2 * DS], f32, tag="lw_xp", name="lw_xp")
                   for _ in range(CBLK)]
            for k in range(CBLK):
                nc.sync.dma_start(lwx[k][:, :],
                                  T["xpw"].ap()[k * 128:(k + 1) * 128, :])
            xdbl = p_xdbl.tile([DTR + 2 * DS, L], f32, tag="xdbl")
            for lh in range(LH):
                psx = p_psx.tile([DTR + 2 * DS, LN], f32, tag="mmx")
                for k in range(CBLK):
                    nc.tensor.matmul(psx[:, :], lwx[k][:, :],
                                     xt_all[k][:, lh * LN:(lh + 1) * LN],
                                     start=(k == 0), stop=(k == CBLK - 1))
                nc.scalar.copy(xdbl[:, lh * LN:(lh + 1) * LN], psx[:, :])
            # stash B/C rows to DRAM (for partition-broadcast reload)
            bcsb = p_misc.tile([2 * DS, L], bcdt, tag="bcsb")
            nc.vector.tensor_copy(bcsb[:, :], xdbl[DTR:, :])
            nc.sync.dma_start(bcs[si].ap(), bcsb[:, :])

            # ---- dt_proj + softplus; w = dt*xt
            dt_all = []
            w_all = []
            for blk in range(NBLK):
                lwd = p_w.tile([DTR, 128], f32, tag="lw_dt")
                nc.sync.dma_start(lwd[:, :],
                                  T["dpw"].ap()[:, blk * 128:(blk + 1) * 128])
                dt_t = p_dt.tile([128, L], f32, tag="dt")
                for lh in range(LH):
                    psd = p_ps.tile([128, LN], f32, tag="mm")
                    nc.tensor.matmul(psd[:, :], lwd[:, :],
                                     xdbl[0:DTR, lh * LN:(lh + 1) * LN],
                                     start=True, stop=True)
                    # softplus(x) = ln(1 + exp(x)); safe: |x| < ~20 here
                    spe = p_ps.tile([128, LN], f32, tag="mm")
                    nc.scalar.activation(spe[:, :], psd[:, :], AF.Exp,
                                         bias=dpb_sb[:, blk:blk + 1], scale=1.0)
                    nc.scalar.activation(dt_t[:, lh * LN:(lh + 1) * LN], spe[:, :],
                                         AF.Ln, bias=1.0, scale=1.0)
                w_t = p_w2.tile([128, L], sdt, tag="w")
                nc.vector.tensor_tensor(w_t[:, :], dt_t[:, :], xt_all[blk][:, :],
                                        OP.mult)
                dt_all.append(dt_t)
                w_all.append(w_t)

            # ---- scan phase: g outer (B/C broadcast reuse), blk inner
            yacc_all = [p_yacc.tile([128, L], f32, tag="yacc", name="yacc")
                        for _ in range(NBLK)]
            bc_base = bcs[si].ap()
            for g in range(NG):
                bcB = p_bc.tile([128, SGRP * L], bcdt, tag="bcB")
                nc.sync.dma_start(
                    bcB[:, :],
                    bass.AP(tensor=bc_base.tensor,
                            offset=bc_base.offset + g * SGRP * L,
                            ap=[[0, 128], [L, SGRP], [1, L]]))
                bcC = p_bc.tile([128, SGRP * L], bcdt, tag="bcC")
                nc.sync.dma_start(
                    bcC[:, :],
                    bass.AP(tensor=bc_base.tensor,
                            offset=bc_base.offset + (DS + g * SGRP) * L,
                            ap=[[0, 128], [L, SGRP], [1, L]]))
                for blk in range(NBLK):
                    dA = p_scan.tile([128, SGRP * L], sdt, tag="dA")
                    for s2 in range(SGRP):
                        s = g * SGRP + s2
                        nc.scalar.activation(
                            dA[:, s2 * L:(s2 + 1) * L], dt_all[blk][:, :], AF.Exp,
                            bias=0.0,
                            scale=A_sb[:, blk * DS + s: blk * DS + s + 1])
                        nc.gpsimd.memset(dA[:, s2 * L:s2 * L + 1], 0.0)
                    dBu = p_scan.tile([128, SGRP * L], sdt, tag="dBu")
                    for s2 in range(SGRP):
                        nc.vector.tensor_tensor(dBu[:, s2 * L:(s2 + 1) * L],
                                                w_all[blk][:, :],
                                                bcB[:, s2 * L:(s2 + 1) * L],
                                                OP.mult)
                    h_t = p_scan.tile([128, SGRP * L], sdt, tag="h")
                    nc.vector.tensor_tensor_scan(h_t[:, :], dA[:, :], dBu[:, :],
                                                 0.0, OP.mult, OP.add)
                    prod = p_scan.tile([128, SGRP * L], sdt, tag="dA")
                    for s2 in range(SGRP):
                        nc.vector.tensor_tensor(prod[:, s2 * L:(s2 + 1) * L],
                                                h_t[:, s2 * L:(s2 + 1) * L],
                                                bcC[:, s2 * L:(s2 + 1) * L],
                                                OP.mult)
                    ya = yacc_all[blk]
                    s0 = 0
                    if g == 0:
                        nc.vector.tensor_tensor(ya[:, :], prod[:, 0:L],
                                                prod[:, L:2 * L], OP.add)
                        s0 = 2
                    for s2 in range(s0, SGRP):
                        nc.vector.tensor_tensor(ya[:, :], ya[:, :],
                                                prod[:, s2 * L:(s2 + 1) * L],
                                                OP.add)

            # ---- gate + out_proj
            yg_all = []
            for blk in range(NBLK):
                y2 = p_misc.tile([128, L], sdt, tag="y2")
                nc.vector.scalar_tensor_tensor(
                    y2[:, :], xt_all[blk][:, :], Dp_sb[:, blk:blk + 1],
                    yacc_all[blk][:, :], OP.mult, OP.add)
                zs = p_misc.tile([128, L], sdt, tag="zs")
                nc.scalar.activation(zs[:, :], z_all[blk][:, :], AF.Tanh,
                                     bias=0.0, scale=0.5)
                zs2 = p_misc.tile([128, L], sdt, tag="zs")
                nc.vector.tensor_scalar(zs2[:, :], zs[:, :], 0.5, 0.5,
                                        OP.mult, OP.add)
                zz = p_misc.tile([128, L], sdt, tag="zz")
                nc.vector.tensor_tensor(zz[:, :], z_all[blk][:, :], zs2[:, :],
                                        OP.mult)
                yg = p_yg.tile([128, L], f32, tag="yg")
                nc.vector.tensor_tensor(yg[:, :], y2[:, :], zz[:, :], OP.mult)
                yg_all.append(yg)

            for i in range(4):
                lwo = []
                for k in range(NBLK):
                    t_ = p_w.tile([128, 128], f32, tag="lw_op", name="lw_op")
                    nc.sync.dma_start(
                        t_[:, :], T["opw"].ap()[k * 128:(k + 1) * 128,
                                                i * 128:(i + 1) * 128])
                    lwo.append(t_)
                yo = p_yo.tile([128, L], f32, tag="yo")
                for lh in range(LH):
                    pso = p_ps.tile([128, LN], f32, tag="mm")
                    for k in range(NBLK):
                        nc.tensor.matmul(pso[:, :], lwo[k][:, :],
                                         yg_all[k][:, lh * LN:(lh + 1) * LN],
                                         start=(k == 0), stop=(k == NBLK - 1))
                    nc.scalar.copy(yo[:, lh * LN:(lh + 1) * LN], pso[:, :])
                nc.sync.dma_start(outs[si].ap()[i * 128:(i + 1) * 128, :],
                                  yo[:, :])

    nc.compile()
    return nc


def _get_program():
    if "nc" not in _CACHE:
        _CACHE["nc"] = _build_program()
    return _CACHE["nc"]


def make_in_maps(inputs):
    """Per-core input maps + metadata; shared by kernel() and tests."""
    inputs = {k: np.asarray(v) for k, v in inputs.items()}
    perms = _direction_perms()
    in_maps = []
    metas = []
    for c in range(NCORE):
        m = {}
        cm = []
        for si in range(NSLOT):
            s = c * NSLOT + si            # global slot 0..23
            u, h = s // 2, s % 2
            p = _slot_params(u, h, inputs, perms)
            cm.append(p["meta"])
            for key in ("x", "ipw", "cdiag", "cb", "xpw", "dpw", "dpb", "A",
                        "Dp", "opw"):
                m[f"{key}_{si}"] = p[key]
        in_maps.append(m)
        metas.append(cm)
    return in_maps, metas, perms


def assemble(results, metas, perms):
    acc = np.zeros((B, DM, L), dtype=np.float64)
    for c in range(NCORE):
        for si in range(NSLOT):
            d, b, h = metas[c][si]
            y = results[c][f"yout_{si}"]       # (DM, L)
            acc[b][:, perms[d]] += y
    return (acc / 6.0).astype(np.float32).reshape(B, DM, D, H, W)


# ---------------------------------------------------------------- entry point
def kernel(**inputs):
    global LAST_RESULTS
    from concourse.bass_utils import run_bass_kernel_spmd

    in_maps, metas, perms = make_in_maps(inputs)
    nc = _get_program()
    res = run_bass_kernel_spmd(nc, in_maps, core_ids=list(range(NCORE)))
    LAST_RESULTS = res
    return assemble(res.results, metas, perms)


# ---------------------------------------------------------------- benchmarking
def _sharded_fn(nc, in_maps):
    """Mirror bass2jax.run_bass_via_pjrt's multi-core path, without donation,
    returning (fn, device_resident_args, out_names, out_avals)."""
    import jax
    import concourse.mybir as mybir
    from jax.sharding import Mesh, PartitionSpec, NamedSharding
    from jax.experimental.shard_map import shard_map
    from concourse import bass2jax
    from concourse.bass2jax import _bass_exec_p, install_neuronx_cc_hook

    install_neuronx_cc_hook()
    from concourse.bass2jax import partition_id_tensor

    part_name = nc.partition_id_tensor.name if nc.partition_id_tensor else None
    in_names, out_names, out_avals, zero_outs = [], [], [], []
    for alloc in nc.m.functions[0].allocations:
        if not isinstance(alloc, mybir.MemoryLocationSet):
            continue
        name = alloc.memorylocations[0].name
        if alloc.kind == "ExternalInput":
            if name != part_name:
                in_names.append(name)
        elif alloc.kind == "ExternalOutput":
            out_names.append(name)
            shape = tuple(alloc.tensor_shape)
            dtype = mybir.dt.np(alloc.dtype)
            out_avals.append(jax.core.ShapedArray(shape, dtype))
            zero_outs.append(np.zeros(shape, dtype))
    n_params = len(in_names)
    all_names = in_names + out_names
    if part_name is not None:
        all_names = all_names + [part_name]

    def _body(*args):
        operands = list(args)
        if part_name is not None:
            operands.append(partition_id_tensor())
        outs = _bass_exec_p.bind(
            *operands,
            out_avals=tuple(out_avals),
            in_names=tuple(all_names),
            out_names=tuple(out_names),
            lowering_input_output_aliases=(),
            sim_require_finite=True,
            sim_require_nnan=True,
            nc=nc,
        )
        return tuple(outs)

    devices = jax.devices()[:NCORE]
    mesh = Mesh(np.asarray(devices), ("core",))
    spec = PartitionSpec("core")
    fn = jax.jit(shard_map(_body, mesh=mesh,
                           in_specs=(spec,) * (n_params + len(out_names)),
                           out_specs=(spec,) * len(out_names),
                           check_rep=False), keep_unused=True)
    sh = NamedSharding(mesh, spec)
    per_core = [[np.asarray(m[n]) for n in in_names] for m in in_maps]
    args = [jax.device_put(
        np.concatenate([per_core[c][i] for c in range(NCORE)], axis=0), sh)
        for i in range(n_params)]
    args += [jax.device_put(
        np.zeros((NCORE * z.shape[0], *z.shape[1:]), z.dtype), sh)
        for z in zero_outs]
    return fn, args, out_names, out_avals


def bench(inputs, iters=20):
    """Return (per_iter_seconds, overhead_seconds, results_list)."""
    import time
    import jax
    in_maps, metas, perms = make_in_maps(inputs)
    nc = _get_program()
    fn, args, out_names, out_avals = _sharded_fn(nc, in_maps)
    out = fn(*args)
    jax.block_until_ready(out)
    t0 = time.perf_counter()
    for _ in range(iters):
        out = fn(*args)
        jax.block_until_ready(out)
    dt = (time.perf_counter() - t0) / iters

    results = [
        {name: np.asarray(out[i]).reshape(NCORE, *out_avals[i].shape)[c]
         for i, name in enumerate(out_names)}
        for c in range(NCORE)
    ]
    return dt, assemble(results, metas, perms), (metas, perms)


def bench_overhead(iters=50):
    """Time a trivial SPMD program to estimate dispatch overhead."""
    import time
    import jax
    import concourse.bacc as bacc
    import concourse.tile as tile
    from concourse import mybir
    from contextlib import ExitStack

    if "nc0" not in _CACHE:
        f32 = mybir.dt.float32
        nc0 = bacc.Bacc("TRN2", target_bir_lowering=False, debug=False,
                        enable_asserts=False, num_devices=1)
        a = nc0.dram_tensor("a", [128, 128], f32, kind="ExternalInput")
        o = nc0.dram_tensor("o", [128, 128], f32, kind="ExternalOutput")
        with tile.TileContext(nc0) as tc, ExitStack() as ctx:
            p = ctx.enter_context(tc.tile_pool(name="p", bufs=1))
            t = p.tile([128, 128], f32, tag="t")
            nc0.sync.dma_start(t[:, :], a.ap())
            nc0.sync.dma_start(o.ap(), t[:, :])
        nc0.compile()
        _CACHE["nc0"] = nc0
    nc0 = _CACHE["nc0"]
    in_maps = [{"a": np.zeros((128, 128), np.float32)} for _ in range(NCORE)]
    fn, args, _, _ = _sharded_fn(nc0, in_maps)
    out = fn(*args)
    jax.block_until_ready(out)
    t0 = time.perf_counter()
    for _ in range(iters):
        out = fn(*args)
        jax.block_until_ready(out)
    return (time.perf_counter() - t0) / iters


# revision 25
# speedup vs baseline: 2.6318x; 2.6318x over previous
"""Trainium2 Bass kernel for the 6-direction vision-Mamba block.

Sharding: 24 uniform slots = (6 directions x 2 batch) x (2 halves of d_inner),
3 slots per core on 8 cores. Host pre-permutes channels per slot so the device
program is identical across cores (SPMD); each slot computes the full in_proj /
conv / x_proj (needed for the full-d_inner contraction feeding dt/B/C) and the
scan + out_proj for its 512-channel half.  Host applies the direction
permutations, sums the 24 partial out_proj results and divides by 6.
"""

import os
import sys

for _p in ("/opt/trn_rl_repo",):
    if os.path.isdir(_p) and _p not in sys.path:
        sys.path.insert(0, _p)

import numpy as np

# ---------------------------------------------------------------- constants
DM = 512       # d_model
DI = 1024      # d_inner
DS = 16        # d_state
DTR = 32       # dt_rank
DCONV = 4
B, D, H, W = 2, 10, 10, 10
L = D * H * W  # 1000
NCORE = 8
NSLOT = 3          # slots per core
NCH = 512          # scan channels per slot
NBLK = NCH // 128  # 4 scan blocks per slot
CBLK = DI // 128   # 8 conv blocks (full d_inner)
LH = 2             # l-halves for matmul N=500
LN = L // LH       # 500

# tuning knobs
SGRP = 2            # states per scan group (16/SGRP groups)
NG = DS // SGRP
STREAM_FP32 = False  # dA/dBu/h/prod/w in fp32 instead of bf16
BC_FP32 = False      # B_rep/C_rep dtype

_CACHE = {}
LAST_RESULTS = None


# ---------------------------------------------------------------- host: perms
def _direction_perms():
    """perm[i][l] = canonical flat (D,H,W) index of the l-th token of dir i."""
    idx = np.arange(L, dtype=np.int64).reshape(D, H, W)
    p1 = idx.reshape(-1)
    p2 = idx.transpose(0, 2, 1).reshape(-1)          # (d, w, h) order
    p3 = np.rot90(idx, 1, axes=(0, 2)).reshape(-1)   # rot90 in (D, W)
    return [p1, p2, p3, p1[::-1].copy(), p2[::-1].copy(), p3[::-1].copy()]


def _slot_params(u, h, inputs, perms):
    """Host-side tensors for one slot (unit u = (dir, batch), half h)."""
    d = u // 2
    b = u % 2
    my = np.arange(h * NCH, (h + 1) * NCH)
    other = np.arange((1 - h) * NCH, (2 - h) * NCH)
    chperm = np.concatenate([my, other])          # my half first

    x = inputs["x"].reshape(B, DM, L)
    x_seq = np.ascontiguousarray(x[b][:, perms[d]], dtype=np.float32)

    ipw = inputs["in_proj_w"][d]                  # (2*DI, DM)
    ipw_sl = np.concatenate([ipw[chperm], ipw[DI + my]], axis=0)   # (1536, DM)
    ipw_t = np.ascontiguousarray(ipw_sl.T, dtype=np.float32)       # (512, 1536)

    cw = inputs["conv_w"][d][chperm, 0, :]        # (1024, 4)
    cdiag = np.zeros((CBLK, DCONV, 128, 128), dtype=np.float32)
    r = np.arange(128)
    for blk in range(CBLK):
        for k in range(DCONV):
            cdiag[blk, k, r, r] = cw[blk * 128 + r, k]
    cb = inputs["conv_b"][d][chperm].reshape(CBLK, 128).T          # (128, 8)
    cb = np.ascontiguousarray(cb, dtype=np.float32)

    xpw = inputs["x_proj_w"][d]                   # (64, DI)
    xpw_t = np.ascontiguousarray(xpw[:, chperm].T, dtype=np.float32)  # (1024, 64)

    dpw = inputs["dt_proj_w"][d][my]              # (512, 32)
    dpw_t = np.ascontiguousarray(dpw.T, dtype=np.float32)             # (32, 512)
    dpb = inputs["dt_proj_b"][d][my].reshape(NBLK, 128).T
    dpb = np.ascontiguousarray(dpb, dtype=np.float32)                 # (128, 4)

    A = -np.exp(inputs["A_log"][d][my])           # (512, 16)
    A_sb = A.reshape(NBLK, 128, DS).transpose(1, 0, 2).reshape(128, NBLK * DS)
    A_sb = np.ascontiguousarray(A_sb, dtype=np.float32)               # (128, 64)

    Dp = inputs["D_param"][d][my].reshape(NBLK, 128).T
    Dp = np.ascontiguousarray(Dp, dtype=np.float32)                   # (128, 4)

    opw = inputs["out_proj_w"][d]                 # (DM, DI)
    opw_t = np.ascontiguousarray(opw[:, my].T, dtype=np.float32)      # (512, 512)

    return dict(x=x_seq, ipw=ipw_t, cdiag=cdiag, cb=cb, xpw=xpw_t,
                dpw=dpw_t, dpb=dpb, A=A_sb, Dp=Dp, opw=opw_t,
                meta=(d, b, h))


# ---------------------------------------------------------------- device build
def _build_program(nslot=NSLOT):
    import concourse.bass as bass
    import concourse.bacc as bacc
    import concourse.tile as tile
    from concourse import mybir
    from contextlib import ExitStack

    f32 = mybir.dt.float32
    sdt = f32 if STREAM_FP32 else mybir.dt.bfloat16
    bcdt = f32 if BC_FP32 else mybir.dt.bfloat16
    AF = mybir.ActivationFunctionType
    OP = mybir.AluOpType

    nc = bacc.Bacc("TRN2", target_bir_lowering=False, debug=False,
                   enable_asserts=False, num_devices=1)

    ins = []
    outs = []
    bcs = []
    for si in range(nslot):
        t = {
            "x": nc.dram_tensor(f"x_{si}", [DM, L], f32, kind="ExternalInput"),
            "ipw": nc.dram_tensor(f"ipw_{si}", [DM, DI + NCH], f32, kind="ExternalInput"),
            "cdiag": nc.dram_tensor(f"cdiag_{si}", [CBLK, DCONV, 128, 128], f32, kind="ExternalInput"),
            "cb": nc.dram_tensor(f"cb_{si}", [128, CBLK], f32, kind="ExternalInput"),
            "xpw": nc.dram_tensor(f"xpw_{si}", [DI, DTR + 2 * DS], f32, kind="ExternalInput"),
            "dpw": nc.dram_tensor(f"dpw_{si}", [DTR, NCH], f32, kind="ExternalInput"),
            "dpb": nc.dram_tensor(f"dpb_{si}", [128, NBLK], f32, kind="ExternalInput"),
            "A": nc.dram_tensor(f"A_{si}", [128, NBLK * DS], f32, kind="ExternalInput"),
            "Dp": nc.dram_tensor(f"Dp_{si}", [128, NBLK], f32, kind="ExternalInput"),
            "opw": nc.dram_tensor(f"opw_{si}", [NCH, DM], f32, kind="ExternalInput"),
        }
        ins.append(t)
        outs.append(nc.dram_tensor(f"yout_{si}", [DM, L], f32, kind="ExternalOutput"))
        bcs.append(nc.dram_tensor(f"bc_{si}", [2 * DS, L], bcdt))  # Internal

    with tile.TileContext(nc) as tc, ExitStack() as ctx:
        p_x = ctx.enter_context(tc.tile_pool(name="p_x", bufs=4))
        p_w = ctx.enter_context(tc.tile_pool(name="p_w", bufs=4))
        p_xcp = ctx.enter_context(tc.tile_pool(name="p_xcp", bufs=2))
        p_xt = ctx.enter_context(tc.tile_pool(name="p_xt", bufs=8))
        p_z = ctx.enter_context(tc.tile_pool(name="p_z", bufs=4))
        p_dt = ctx.enter_context(tc.tile_pool(name="p_dt", bufs=4))
        p_w2 = ctx.enter_context(tc.tile_pool(name="p_w2", bufs=4))
        p_xdbl = ctx.enter_context(tc.tile_pool(name="p_xdbl", bufs=1))
        p_bc = ctx.enter_context(tc.tile_pool(name="p_bc", bufs=2))
        p_scan = ctx.enter_context(tc.tile_pool(name="p_scan", bufs=2))
        p_yacc = ctx.enter_context(tc.tile_pool(name="p_yacc", bufs=4))
        p_misc = ctx.enter_context(tc.tile_pool(name="p_misc", bufs=2))
        p_yo = ctx.enter_context(tc.tile_pool(name="p_yo", bufs=1))
        p_yg = ctx.enter_context(tc.tile_pool(name="p_yg", bufs=4))
        p_const = ctx.enter_context(tc.tile_pool(name="p_const", bufs=4))
        p_ps = ctx.enter_context(tc.tile_pool(name="p_ps", bufs=3, space="PSUM"))
        p_psx = ctx.enter_context(tc.tile_pool(name="p_psx", bufs=2, space="PSUM"))

        for si in range(nslot):
            T = ins[si]
            # ---- constants
            cb_sb = p_const.tile([128, CBLK], f32, tag="cb")
            nc.sync.dma_start(cb_sb[:, :], T["cb"].ap())
            dpb_sb = p_const.tile([128, NBLK], f32, tag="dpb")
            nc.sync.dma_start(dpb_sb[:, :], T["dpb"].ap())
            A_sb = p_const.tile([128, NBLK * DS], f32, tag="A")
            nc.sync.dma_start(A_sb[:, :], T["A"].ap())
            Dp_sb = p_const.tile([128, NBLK], f32, tag="Dp")
            nc.sync.dma_start(Dp_sb[:, :], T["Dp"].ap())
            cbh_sb = p_const.tile([128, CBLK], f32, tag="cbh")
            nc.vector.tensor_scalar_mul(cbh_sb[:, :], cb_sb[:, :], 0.5)

            # ---- load x (4 k-tiles)
            x_sb = []
            for k in range(4):
                xin = p_x.tile([128, L], f32, tag="x")
                nc.sync.dma_start(xin[:, :], T["x"].ap()[k * 128:(k + 1) * 128, :])
                x_sb.append(xin)

            # ---- in_proj xc rows (8 blocks) interleaved with depthwise conv
            xt_all = []
            for blk in range(CBLK):
                xcp = p_xcp.tile([128, L + 3], f32, tag="xcp")
                nc.gpsimd.memset(xcp[:, 0:3], 0.0)
                lw = [p_w.tile([128, 128], f32, tag="lw_ip", name="lw_ip") for _ in range(4)]
                for k in range(4):
                    nc.sync.dma_start(
                        lw[k][:, :],
                        T["ipw"].ap()[k * 128:(k + 1) * 128,
                                      blk * 128:(blk + 1) * 128])
                for lh in range(LH):
                    ps = p_ps.tile([128, LN], f32, tag="mm")
                    for k in range(4):
                        nc.tensor.matmul(ps[:, :], lw[k][:, :],
                                         x_sb[k][:, lh * LN:(lh + 1) * LN],
                                         start=(k == 0), stop=(k == 3))
                    nc.vector.tensor_copy(xcp[:, 3 + lh * LN: 3 + (lh + 1) * LN],
                                          ps[:, :])
                # depthwise conv via 4 diagonal matmuls + fused bias+silu
                xt_t = p_xt.tile([128, L], f32, tag="xt")
                lwc = [p_w.tile([128, 128], f32, tag="lw_cv", name="lw_cv") for _ in range(4)]
                for k in range(4):
                    nc.sync.dma_start(lwc[k][:, :], T["cdiag"].ap()[blk, k])
                for lh in range(LH):
                    psc = p_ps.tile([128, LN], f32, tag="mm")
                    for k in range(4):
                        nc.tensor.matmul(psc[:, :], lwc[k][:, :],
                                         xcp[:, lh * LN + k: lh * LN + k + LN],
                                         start=(k == 0), stop=(k == 3))
                    # silu(v+cb) = (v+cb)*(0.5 + 0.5*tanh((v+cb)/2))
                    cvt = p_misc.tile([128, LN], sdt, tag="cvt")
                    nc.scalar.activation(cvt[:, :], psc[:, :], AF.Tanh,
                                         bias=cbh_sb[:, blk:blk + 1], scale=0.5)
                    cvt2 = p_misc.tile([128, LN], sdt, tag="cvt2")
                    nc.vector.tensor_scalar(cvt2[:, :], cvt[:, :], 0.5, 0.5,
                                            OP.mult, OP.add)
                    nc.vector.scalar_tensor_tensor(
                        xt_t[:, lh * LN:(lh + 1) * LN], psc[:, :],
                        cb_sb[:, blk:blk + 1], cvt2[:, :], OP.add, OP.mult)
                xt_all.append(xt_t)

            # ---- in_proj z rows (4 blocks, my half)
            z_all = []
            for zb in range(NBLK):
                i = CBLK + zb
                lw = [p_w.tile([128, 128], f32, tag="lw_ip", name="lw_ip") for _ in range(4)]
                for k in range(4):
                    nc.sync.dma_start(
                        lw[k][:, :],
                        T["ipw"].ap()[k * 128:(k + 1) * 128,
                                      i * 128:(i + 1) * 128])
                z_t = p_z.tile([128, L], sdt, tag="z")
                for lh in range(LH):
                    ps = p_ps.tile([128, LN], f32, tag="mm")
                    for k in range(4):
                        nc.tensor.matmul(ps[:, :], lw[k][:, :],
                                         x_sb[k][:, lh * LN:(lh + 1) * LN],
                                         start=(k == 0), stop=(k == 3))
                    nc.scalar.copy(z_t[:, lh * LN:(lh + 1) * LN], ps[:, :])
                z_all.append(z_t)

            # ---- x_proj (contract full d_inner)
            lwx = [p_w.tile([128, DTR + 2 * DS], f32, tag="lw_xp", name="lw_xp")
                   for _ in range(CBLK)]
            for k in range(CBLK):
                nc.sync.dma_start(lwx[k][:, :],
                                  T["xpw"].ap()[k * 128:(k + 1) * 128, :])
            xdbl = p_xdbl.tile([DTR + 2 * DS, L], f32, tag="xdbl")
            for lh in range(LH):
                psx = p_psx.tile([DTR + 2 * DS, LN], f32, tag="mmx")
                for k in range(CBLK):
                    nc.tensor.matmul(psx[:, :], lwx[k][:, :],
                                     xt_all[k][:, lh * LN:(lh + 1) * LN],
                                     start=(k == 0), stop=(k == CBLK - 1))
                nc.scalar.copy(xdbl[:, lh * LN:(lh + 1) * LN], psx[:, :])
            # stash B/C rows to DRAM (for partition-broadcast reload)
            bcsb = p_misc.tile([2 * DS, L], bcdt, tag="bcsb")
            nc.vector.tensor_copy(bcsb[:, :], xdbl[DTR:, :])
            nc.sync.dma_start(bcs[si].ap(), bcsb[:, :])

            # ---- dt_proj + softplus; w = dt*xt
            dt_all = []
            w_all = []
            for blk in range(NBLK):
                lwd = p_w.tile([DTR, 128], f32, tag="lw_dt")
                nc.sync.dma_start(lwd[:, :],
                                  T["dpw"].ap()[:, blk * 128:(blk + 1) * 128])
                dt_t = p_dt.tile([128, L], f32, tag="dt")
                for lh in range(LH):
                    psd = p_ps.tile([128, LN], f32, tag="mm")
                    nc.tensor.matmul(psd[:, :], lwd[:, :],
                                     xdbl[0:DTR, lh * LN:(lh + 1) * LN],
                                     start=True, stop=True)
                    # softplus(x) = ln(1 + exp(x)); safe: |x| < ~20 here
                    spe = p_ps.tile([128, LN], f32, tag="mm")
                    nc.scalar.activation(spe[:, :], psd[:, :], AF.Exp,
                                         bias=dpb_sb[:, blk:blk + 1], scale=1.0)
                    nc.scalar.activation(dt_t[:, lh * LN:(lh + 1) * LN], spe[:, :],
                                         AF.Ln, bias=1.0, scale=1.0)
                w_t = p_w2.tile([128, L], sdt, tag="w")
                nc.vector.tensor_tensor(w_t[:, :], dt_t[:, :], xt_all[blk][:, :],
                                        OP.mult)
                dt_all.append(dt_t)
                w_all.append(w_t)

            # ---- scan phase: g outer (B/C broadcast reuse), blk inner
            yacc_all = [p_yacc.tile([128, L], f32, tag="yacc", name="yacc")
                        for _ in range(NBLK)]
            bc_base = bcs[si].ap()
            for g in range(NG):
                bcB = p_bc.tile([128, SGRP * L], bcdt, tag="bcB")
                nc.sync.dma_start(
                    bcB[:, :],
                    bass.AP(tensor=bc_base.tensor,
                            offset=bc_base.offset + g * SGRP * L,
                            ap=[[0, 128], [L, SGRP], [1, L]]))
                bcC = p_bc.tile([128, SGRP * L], bcdt, tag="bcC")
                nc.sync.dma_start(
                    bcC[:, :],
                    bass.AP(tensor=bc_base.tensor,
                            offset=bc_base.offset + (DS + g * SGRP) * L,
                            ap=[[0, 128], [L, SGRP], [1, L]]))
                for blk in range(NBLK):
                    dA = p_scan.tile([128, SGRP * L], sdt, tag="dA")
                    for s2 in range(SGRP):
                        s = g * SGRP + s2
                        nc.scalar.activation(
                            dA[:, s2 * L:(s2 + 1) * L], dt_all[blk][:, :], AF.Exp,
                            bias=0.0,
                            scale=A_sb[:, blk * DS + s: blk * DS + s + 1])
                        nc.gpsimd.memset(dA[:, s2 * L:s2 * L + 1], 0.0)
                    dBu = p_scan.tile([128, SGRP * L], sdt, tag="dBu")
                    for s2 in range(SGRP):
                        nc.vector.tensor_tensor(dBu[:, s2 * L:(s2 + 1) * L],
                                                w_all[blk][:, :],
                                                bcB[:, s2 * L:(s2 + 1) * L],
                                                OP.mult)
                    h_t = p_scan.tile([128, SGRP * L], sdt, tag="h")
                    nc.vector.tensor_tensor_scan(h_t[:, :], dA[:, :], dBu[:, :],
                                                 0.0, OP.mult, OP.add)
                    prod = p_scan.tile([128, SGRP * L], sdt, tag="dA")
                    for s2 in range(SGRP):
                        nc.vector.tensor_tensor(prod[:, s2 * L:(s2 + 1) * L],
                                                h_t[:, s2 * L:(s2 + 1) * L],
                                                bcC[:, s2 * L:(s2 + 1) * L],
                                                OP.mult)
                    ya = yacc_all[blk]
                    s0 = 0
                    if g == 0:
                        nc.vector.tensor_tensor(ya[:, :], prod[:, 0:L],
                                                prod[:, L:2 * L], OP.add)
                        s0 = 2
                    for s2 in range(s0, SGRP):
                        nc.vector.tensor_tensor(ya[:, :], ya[:, :],
                                                prod[:, s2 * L:(s2 + 1) * L],
                                                OP.add)

            # ---- gate + out_proj
            yg_all = []
            for blk in range(NBLK):
                y2 = p_misc.tile([128, L], sdt, tag="y2")
                nc.vector.scalar_tensor_tensor(
                    y2[:, :], xt_all[blk][:, :], Dp_sb[:, blk:blk + 1],
                    yacc_all[blk][:, :], OP.mult, OP.add)
                zs = p_misc.tile([128, L], sdt, tag="zs")
                nc.scalar.activation(zs[:, :], z_all[blk][:, :], AF.Tanh,
                                     bias=0.0, scale=0.5)
                zs2 = p_misc.tile([128, L], sdt, tag="zs")
                nc.vector.tensor_scalar(zs2[:, :], zs[:, :], 0.5, 0.5,
                                        OP.mult, OP.add)
                zz = p_misc.tile([128, L], sdt, tag="zz")
                nc.vector.tensor_tensor(zz[:, :], z_all[blk][:, :], zs2[:, :],
                                        OP.mult)
                yg = p_yg.tile([128, L], f32, tag="yg")
                nc.vector.tensor_tensor(yg[:, :], y2[:, :], zz[:, :], OP.mult)
                yg_all.append(yg)

            for i in range(4):
                lwo = []
                for k in range(NBLK):
                    t_ = p_w.tile([128, 128], f32, tag="lw_op", name="lw_op")
                    nc.sync.dma_start(
                        t_[:, :], T["opw"].ap()[k * 128:(k + 1) * 128,
                                                i * 128:(i + 1) * 128])
                    lwo.append(t_)
                yo = p_yo.tile([128, L], f32, tag="yo")
                for lh in range(LH):
                    pso = p_ps.tile([128, LN], f32, tag="mm")
                    for k in range(NBLK):
                        nc.tensor.matmul(pso[:, :], lwo[k][:, :],
                                         yg_all[k][:, lh * LN:(lh + 1) * LN],
                                         start=(k == 0), stop=(k == NBLK - 1))
                    nc.scalar.copy(yo[:, lh * LN:(lh + 1) * LN], pso[:, :])
                nc.sync.dma_start(outs[si].ap()[i * 128:(i + 1) * 128, :],
                                  yo[:, :])

    nc.compile()
    return nc


def _get_program(nslot=NSLOT):
    key = f"nc{nslot}"
    if key not in _CACHE:
        _CACHE[key] = _build_program(nslot)
    return _CACHE[key]


def make_in_maps(inputs, nslot=NSLOT):
    """Per-core input maps + metadata; shared by kernel() and tests."""
    inputs = {k: np.asarray(v) for k, v in inputs.items()}
    perms = _direction_perms()
    in_maps = []
    metas = []
    for c in range(NCORE):
        m = {}
        cm = []
        for si in range(nslot):
            s = c * NSLOT + (si % NSLOT)  # global slot 0..23
            u, h = s // 2, s % 2
            p = _slot_params(u, h, inputs, perms)
            cm.append(p["meta"])
            for key in ("x", "ipw", "cdiag", "cb", "xpw", "dpw", "dpb", "A",
                        "Dp", "opw"):
                m[f"{key}_{si}"] = p[key]
        in_maps.append(m)
        metas.append(cm)
    return in_maps, metas, perms


def assemble(results, metas, perms):
    acc = np.zeros((B, DM, L), dtype=np.float64)
    for c in range(NCORE):
        for si in range(NSLOT):
            d, b, h = metas[c][si]
            y = results[c][f"yout_{si}"]       # (DM, L)
            acc[b][:, perms[d]] += y
    return (acc / 6.0).astype(np.float32).reshape(B, DM, D, H, W)


# ---------------------------------------------------------------- entry point
def kernel(**inputs):
    global LAST_RESULTS
    from concourse.bass_utils import run_bass_kernel_spmd

    in_maps, metas, perms = make_in_maps(inputs)
    nc = _get_program()
    res = run_bass_kernel_spmd(nc, in_maps, core_ids=list(range(NCORE)))
    LAST_RESULTS = res
    return assemble(res.results, metas, perms)


# ---------------------------------------------------------------- benchmarking
def _sharded_fn(nc, in_maps):
    """Mirror bass2jax.run_bass_via_pjrt's multi-core path, without donation,
    returning (fn, device_resident_args, out_names, out_avals)."""
    import jax
    import concourse.mybir as mybir
    from jax.sharding import Mesh, PartitionSpec, NamedSharding
    from jax.experimental.shard_map import shard_map
    from concourse import bass2jax
    from concourse.bass2jax import _bass_exec_p, install_neuronx_cc_hook

    install_neuronx_cc_hook()
    from concourse.bass2jax import partition_id_tensor

    part_name = nc.partition_id_tensor.name if nc.partition_id_tensor else None
    in_names, out_names, out_avals, zero_outs = [], [], [], []
    for alloc in nc.m.functions[0].allocations:
        if not isinstance(alloc, mybir.MemoryLocationSet):
            continue
        name = alloc.memorylocations[0].name
        if alloc.kind == "ExternalInput":
            if name != part_name:
                in_names.append(name)
        elif alloc.kind == "ExternalOutput":
            out_names.append(name)
            shape = tuple(alloc.tensor_shape)
            dtype = mybir.dt.np(alloc.dtype)
            out_avals.append(jax.core.ShapedArray(shape, dtype))
            zero_outs.append(np.zeros(shape, dtype))
    n_params = len(in_names)
    all_names = in_names + out_names
    if part_name is not None:
        all_names = all_names + [part_name]

    def _body(*args):
        operands = list(args)
        if part_name is not None:
            operands.append(partition_id_tensor())
        outs = _bass_exec_p.bind(
            *operands,
            out_avals=tuple(out_avals),
            in_names=tuple(all_names),
            out_names=tuple(out_names),
            lowering_input_output_aliases=(),
            sim_require_finite=True,
            sim_require_nnan=True,
            nc=nc,
        )
        return tuple(outs)

    devices = jax.devices()[:NCORE]
    mesh = Mesh(np.asarray(devices), ("core",))
    spec = PartitionSpec("core")
    fn = jax.jit(shard_map(_body, mesh=mesh,
                           in_specs=(spec,) * (n_params + len(out_names)),
                           out_specs=(spec,) * len(out_names),
                           check_rep=False), keep_unused=True)
    sh = NamedSharding(mesh, spec)
    per_core = [[np.asarray(m[n]) for n in in_names] for m in in_maps]
    args = [jax.device_put(
        np.concatenate([per_core[c][i] for c in range(NCORE)], axis=0), sh)
        for i in range(n_params)]
    args += [jax.device_put(
        np.zeros((NCORE * z.shape[0], *z.shape[1:]), z.dtype), sh)
        for z in zero_outs]
    return fn, args, out_names, out_avals


def bench_chain(inputs, nchain=5, iters=5):
    """Time nchain dependency-chained kernel executions per launch; the
    difference vs a 1-chain launch isolates pure kernel exec time."""
    import time
    import jax
    import concourse.mybir as mybir
    from jax.sharding import Mesh, PartitionSpec, NamedSharding
    from jax.experimental.shard_map import shard_map
    from concourse.bass2jax import (_bass_exec_p, install_neuronx_cc_hook,
                                    partition_id_tensor)

    in_maps, metas, perms = make_in_maps(inputs)
    nc = _get_program()
    install_neuronx_cc_hook()
    part_name = nc.partition_id_tensor.name if nc.partition_id_tensor else None
    in_names, out_names, out_avals, zero_outs = [], [], [], []
    for alloc in nc.m.functions[0].allocations:
        if not isinstance(alloc, mybir.MemoryLocationSet):
            continue
        name = alloc.memorylocations[0].name
        if alloc.kind == "ExternalInput":
            if name != part_name:
                in_names.append(name)
        elif alloc.kind == "ExternalOutput":
            out_names.append(name)
            shape = tuple(alloc.tensor_shape)
            dtype = mybir.dt.np(alloc.dtype)
            out_avals.append(jax.core.ShapedArray(shape, dtype))
            zero_outs.append(np.zeros(shape, dtype))
    n_params = len(in_names)
    all_names = in_names + out_names + ([part_name] if part_name else [])

    def _body(*args):
        operands = list(args)
        if part_name is not None:
            operands.append(partition_id_tensor())
        return tuple(_bass_exec_p.bind(
            *operands,
            out_avals=tuple(out_avals),
            in_names=tuple(all_names),
            out_names=tuple(out_names),
            lowering_input_output_aliases=(),
            sim_require_finite=True, sim_require_nnan=True, nc=nc))

    def _chain(n):
        def f(*args):
            a = list(args)
            outs = _body(*a)
            for _ in range(n - 1):
                # feed yout_0 (same shape/dtype as x_0) back in to force
                # sequential execution and defeat CSE
                a = [outs[0]] + a[1:]
                outs = _body(*a)
            return outs
        return f

    devices = jax.devices()[:NCORE]
    mesh = Mesh(np.asarray(devices), ("core",))
    spec = PartitionSpec("core")
    sh = NamedSharding(mesh, spec)
    per_core = [[np.asarray(m[n]) for n in in_names] for m in in_maps]
    args = [jax.device_put(
        np.concatenate([per_core[c][i] for c in range(NCORE)], axis=0), sh)
        for i in range(n_params)]
    args += [jax.device_put(
        np.zeros((NCORE * z.shape[0], *z.shape[1:]), z.dtype), sh)
        for z in zero_outs]

    times = {}
    for n in (1, nchain):
        fn = jax.jit(shard_map(_chain(n), mesh=mesh,
                               in_specs=(spec,) * len(args),
                               out_specs=(spec,) * len(out_names),
                               check_rep=False), keep_unused=True)
        out = fn(*args)
        jax.block_until_ready(out)
        best = float("inf")
        for _ in range(iters):
            t0 = time.perf_counter()
            out = fn(*args)
            jax.block_until_ready(out)
            best = min(best, time.perf_counter() - t0)
        times[n] = best
    per_exec = (times[nchain] - times[1]) / (nchain - 1)
    return per_exec, times


def bench(inputs, iters=20):
    """Return (per_iter_seconds, overhead_seconds, results_list)."""
    import time
    import jax
    in_maps, metas, perms = make_in_maps(inputs)
    nc = _get_program()
    fn, args, out_names, out_avals = _sharded_fn(nc, in_maps)
    out = fn(*args)
    jax.block_until_ready(out)
    dt = float("inf")
    for _ in range(iters):
        t0 = time.perf_counter()
        out = fn(*args)
        jax.block_until_ready(out)
        dt = min(dt, time.perf_counter() - t0)

    results = [
        {name: np.asarray(out[i]).reshape(NCORE, *out_avals[i].shape)[c]
         for i, name in enumerate(out_names)}
        for c in range(NCORE)
    ]
    return dt, assemble(results, metas, perms), (metas, perms)


def bench_overhead(iters=50):
    """Time a trivial SPMD program to estimate dispatch overhead."""
    import time
    import jax
    import concourse.bacc as bacc
    import concourse.tile as tile
    from concourse import mybir
    from contextlib import ExitStack

    if "nc0" not in _CACHE:
        f32 = mybir.dt.float32
        nc0 = bacc.Bacc("TRN2", target_bir_lowering=False, debug=False,
                        enable_asserts=False, num_devices=1)
        a = nc0.dram_tensor("a", [128, 128], f32, kind="ExternalInput")
        o = nc0.dram_tensor("o", [128, 128], f32, kind="ExternalOutput")
        with tile.TileContext(nc0) as tc, ExitStack() as ctx:
            p = ctx.enter_context(tc.tile_pool(name="p", bufs=1))
            t = p.tile([128, 128], f32, tag="t")
            nc0.sync.dma_start(t[:, :], a.ap())
            nc0.sync.dma_start(o.ap(), t[:, :])
        nc0.compile()
        _CACHE["nc0"] = nc0
    nc0 = _CACHE["nc0"]
    in_maps = [{"a": np.zeros((128, 128), np.float32)} for _ in range(NCORE)]
    fn, args, _, _ = _sharded_fn(nc0, in_maps)
    out = fn(*args)
    jax.block_until_ready(out)
    best = float("inf")
    for _ in range(iters):
        t0 = time.perf_counter()
        out = fn(*args)
        jax.block_until_ready(out)
        best = min(best, time.perf_counter() - t0)
    return best
